# revision 4
# baseline (speedup 1.0000x reference)
"""LSTM warmup+autoregressive-decode kernel for 8 Trainium2 NeuronCores.

Strategy (data-parallel over batch — zero recurring collectives):
  - Each core owns 64 of the 512 batch rows and computes the FULL LSTM for
    them. The per-step h AllGather of the old tensor-parallel design is gone;
    on this runtime every collective call costs ~0.6-0.9ms, which made the
    old design's 148 collective calls ~127ms of pure collective overhead.
  - Weights are shipped SHARDED (1/8 of the rows per core) and assembled
    on-device with ONE AllGather into a shared-DRAM buffer, then streamed
    from HBM every step (they exceed SBUF capacity).
  - Layout: batch on the stationary side (lhsT = x^T/h^T k-tiles [128, 64]),
    weights as the 512-wide moving operand -> full fp16 matmul rate; z lives
    as [64 batch, 512 gate-cols] PSUM tiles.
  - Gate columns are host-permuted to [u-tile][i|f|g|o][512] so the four
    gates of one u-tile arrive in consecutive PSUM tiles (4 live banks).
  - Decode folds the feedback path on host: z = h @ (rec + dense_w @ kernel),
    so decode streams one 32MB matrix + 8MB dense per step.
  - h is re-transposed each step with 16 PE-transposes (hidden under the
    weight stream); predictions are written untransposed [64, 512] tiles.

kernel(**inputs) takes the full unsharded inputs and returns [B, OUT, F].
"""

import sys, time as _time

for _p in ("/opt/trn_rl_repo", "/root/.axon_site/_ro/trn_rl_repo"):
    if _p not in sys.path:
        sys.path.insert(0, _p)

import os

import numpy as np

import concourse.bass as bass
import concourse.mybir as mybir
import concourse.tile as tile
from concourse import bacc
from concourse.bass import ts
from concourse.bass_utils import run_bass_kernel_spmd

B, T, F, U = 512, 48, 2048, 2048
OUT_STEPS = 24
W = 8  # cores
BL = B // W  # 64 batch rows per core
KT = F // 128  # 16 k-tiles over the x/h feature dim
UT = U // 512  # 4 u-tiles of 512 features
GC = 26624  # gathered weight cols: Kp 8192 | Rp 8192 | Wd 8192 | dw 2048
OFF_K, OFF_R, OFF_D, OFF_W = 0, 8192, 16384, 24576
FP16 = mybir.dt.float16
FP32 = mybir.dt.float32
AF = mybir.ActivationFunctionType

_last_results = {"exec_time_ns": None}


def build_nc(t_warm=T, t_dec=OUT_STEPS - 1, emit_bias=False, emit_bdec=False,
             emit_db=False):
    nc = bacc.Bacc("TRN2", target_bir_lowering=False, debug=False, num_devices=W)

    wsh_in = nc.dram_tensor("wsh", [F // W, GC], FP16, kind="ExternalInput")
    x_in = nc.dram_tensor("x_t", [t_warm, 128, KT, BL], FP16, kind="ExternalInput")
    eye_in = nc.dram_tensor("eye", [BL, BL], FP16, kind="ExternalInput")
    if emit_bias:
        bias_in = nc.dram_tensor("bias_bc", [BL, 4 * U], FP32, kind="ExternalInput")
    if emit_bdec:
        bdec_in = nc.dram_tensor("bdec_bc", [BL, 4 * U], FP32, kind="ExternalInput")
    if emit_db:
        db_in = nc.dram_tensor("db_bc", [BL, F], FP32, kind="ExternalInput")
    p_out = nc.dram_tensor(
        "preds", [t_dec + 1, UT, BL, 512], FP16, kind="ExternalOutput"
    )

    with tile.TileContext(nc) as tc:
        with (
            tc.tile_pool(name="wstr", bufs=6) as wstr,
            tc.tile_pool(name="cpool", bufs=1) as cpool,
            tc.tile_pool(name="hbufs", bufs=2) as hbufs,
            tc.tile_pool(name="xbufs", bufs=2) as xbufs,
            tc.tile_pool(name="gtmp", bufs=2) as gtmp,
            tc.tile_pool(name="outp", bufs=4) as outp,
            tc.tile_pool(name="zps", bufs=3, space="PSUM") as zps,
            tc.tile_pool(name="tps", bufs=2, space="PSUM") as tps,
            tc.tile_pool(name="pps", bufs=2, space="PSUM") as pps,
            tc.tile_pool(name="agin", bufs=1, space="DRAM") as agin,
            tc.tile_pool(name="agout", bufs=1, space="DRAM") as agout,
        ):
            # --- one-time weight assembly: ship shards, AllGather once ---
            wb = agin.tile([F // W, GC], FP16, tag="wagin")
            nc.sync.dma_start(wb[:], wsh_in[:])
            wg = agout.tile([F, GC], FP16, addr_space="Shared", tag="wagout")
            if os.environ.get("SKIP_AG"):
                nc.sync.dma_start(wg[0 : F // W, :], wb[:])
            else:
                nc.gpsimd.collective_compute(
                    "AllGather",
                    mybir.AluOpType.bypass,
                    replica_groups=[list(range(W))],
                    ins=[wb[:].opt()],
                    outs=[wg[:].opt()],
                )

            def wtile(off, m):
                """Weight stream source: [128, KT, 512] view of gathered cols
                [off+512m, off+512(m+1>)."""
                return wg[:, off + 512 * m : off + 512 * (m + 1)].rearrange(
                    "(kt p) n -> p kt n", p=128
                )

            eye_sb = cpool.tile([BL, BL], FP16, tag="eye")
            nc.sync.dma_start(eye_sb[:], eye_in[:])
            bias_sb = None
            bdec_sb = None
            db_sb = None
            if emit_bias:
                bias_sb = cpool.tile([BL, 4 * U], FP32, tag="biasbc")
                nc.sync.dma_start(bias_sb[:], bias_in[:])
            if emit_bdec:
                bdec_sb = cpool.tile([BL, 4 * U], FP32, tag="bdecbc")
                nc.sync.dma_start(bdec_sb[:], bdec_in[:])
            if emit_db:
                db_sb = cpool.tile([BL, F], FP32, tag="dbbc")
                nc.sync.dma_start(db_sb[:], db_in[:])

            c_st = [
                cpool.tile([BL, 512], FP32, tag=f"c{u}", name=f"c_st{u}")
                for u in range(UT)
            ]
            for cs in c_st:
                nc.vector.memset(cs[:], 0.0)

            def lstm_step(emit_zp, hT_prev, step_bias):
                """One LSTM step. emit_zp(ut, g) -> PSUM tile [BL, 512] with
                the matmul accumulation for that gate slice. Returns hT_next
                ([128, KT, BL] fp16)."""
                hT_next = hbufs.tile([128, KT, BL], FP16, tag="hT")
                for ut in range(UT):
                    acts = []
                    for g, (fn, tag) in enumerate(
                        [
                            (AF.Sigmoid, "si"),
                            (AF.Sigmoid, "sf"),
                            (AF.Tanh, "tg"),
                            (AF.Sigmoid, "so"),
                        ]
                    ):
                        zp = emit_zp(ut, g)
                        src = zp[:]
                        if step_bias is not None:
                            zb = gtmp.tile([BL, 512], FP32, tag="zb")
                            nc.vector.tensor_tensor(
                                zb[:],
                                zp[:],
                                step_bias[:, ts(ut * 4 + g, 512)],
                                mybir.AluOpType.add,
                            )
                            src = zb[:]
                        a = gtmp.tile([BL, 512], FP16, tag=tag)
                        nc.scalar.activation(a[:], src, fn)
                        acts.append(a)
                    si, sf, tg, so = acts
                    t1 = gtmp.tile([BL, 512], FP32, tag="t1")
                    t2 = gtmp.tile([BL, 512], FP32, tag="t2")
                    nc.vector.tensor_tensor(
                        t1[:], sf[:], c_st[ut][:], mybir.AluOpType.mult
                    )
                    nc.vector.tensor_tensor(t2[:], si[:], tg[:], mybir.AluOpType.mult)
                    nc.vector.tensor_tensor(
                        c_st[ut][:], t1[:], t2[:], mybir.AluOpType.add
                    )
                    tcs = gtmp.tile([BL, 512], FP16, tag="tc")
                    nc.scalar.activation(tcs[:], c_st[ut][:], AF.Tanh)
                    hn = gtmp.tile([BL, 512], FP16, tag="hn")
                    nc.vector.tensor_tensor(hn[:], so[:], tcs[:], mybir.AluOpType.mult)
                    # transpose h_new [64, 512] -> 4 tiles of [128, 64]
                    for j in range(4):
                        tp = tps.tile([128, BL], FP16, tag="tp")
                        nc.tensor.transpose(
                            tp[:], hn[:, ts(j, 128)], eye_sb[:]
                        )
                        nc.scalar.activation(
                            hT_next[:, ut * 4 + j, :], tp[:], AF.Identity
                        )
                return hT_next

            def emit_pred(hT, t_idx):
                """pred = h @ dense_w (+ db): 4 tiles of [64, 512] to DRAM."""
                for ft in range(UT):
                    dstr = wstr.tile([128, KT, 512], FP16, tag="wstr", name=f"dw{t_idx}_{ft}")
                    nc.sync.dma_start(dstr[:], wtile(OFF_W, ft))
                    pp = pps.tile([BL, 512], FP32, tag="pp")
                    for kt in range(KT):
                        nc.tensor.matmul(
                            pp[:],
                            hT[:, kt, :],
                            dstr[:, kt, :],
                            start=(kt == 0),
                            stop=(kt == KT - 1),
                        )
                    po = outp.tile([BL, 512], FP16, tag="po")
                    if db_sb is not None:
                        pb = gtmp.tile([BL, 512], FP32, tag="pb")
                        nc.vector.tensor_tensor(
                            pb[:], pp[:], db_sb[:, ts(ft, 512)], mybir.AluOpType.add
                        )
                        nc.scalar.activation(po[:], pb[:], AF.Identity)
                    else:
                        nc.scalar.activation(po[:], pp[:], AF.Identity)
                    nc.sync.dma_start(p_out[t_idx, ft], po[:])

            # ---------------- warmup ----------------
            hT = None
            for t in range(t_warm):
                xcur = xbufs.tile([128, KT, BL], FP16, tag="xT")
                nc.sync.dma_start(xcur[:], x_in[t])

                def z_mm(ut, g, xcur=xcur, hT=hT, first=(t == 0)):
                    m = ut * 4 + g
                    kstr = wstr.tile([128, KT, 512], FP16, tag="wstr")
                    nc.sync.dma_start(kstr[:], wtile(OFF_K, m))
                    zp = zps.tile([BL, 512], FP32, tag="z")
                    if not first:
                        rstr = wstr.tile([128, KT, 512], FP16, tag="wstr")
                        nc.sync.dma_start(rstr[:], wtile(OFF_R, m))
                    for kt in range(KT):
                        nc.tensor.matmul(
                            zp[:],
                            xcur[:, kt, :],
                            kstr[:, kt, :],
                            start=(kt == 0),
                            stop=first and (kt == KT - 1),
                        )
                    if not first:
                        for kt in range(KT):
                            nc.tensor.matmul(
                                zp[:],
                                hT[:, kt, :],
                                rstr[:, kt, :],
                                start=False,
                                stop=(kt == KT - 1),
                            )
                    return zp

                hT = lstm_step(z_mm, hT, bias_sb[:] if emit_bias else None)

            emit_pred(hT, 0)

            # ---------------- decode ----------------
            for t in range(t_dec):

                def z_mm(ut, g, hT=hT):
                    m = ut * 4 + g
                    dstr = wstr.tile([128, KT, 512], FP16, tag="wstr")
                    nc.sync.dma_start(dstr[:], wtile(OFF_D, m))
                    zp = zps.tile([BL, 512], FP32, tag="z")
                    for kt in range(KT):
                        nc.tensor.matmul(
                            zp[:],
                            hT[:, kt, :],
                            dstr[:, kt, :],
                            start=(kt == 0),
                            stop=(kt == KT - 1),
                        )
                    return zp

                hT = lstm_step(z_mm, hT, bdec_sb[:] if emit_bdec else None)
                emit_pred(hT, t + 1)

    nc.compile()
    return nc


def _perm_cols():
    # permuted gate cols: [u-tile][i|f|g|o][512]
    m = np.arange(4 * U)
    ut, rem = m // (4 * 512), m % (4 * 512)
    g, j = rem // 512, rem % 512
    return g * U + ut * 512 + j


def _prep_inputs(inputs, kernel, rec_kernel, bias, dense_w, dense_b, t_warm):
    x = np.asarray(inputs, np.float32)
    kern = np.asarray(kernel, np.float32)
    rec = np.asarray(rec_kernel, np.float32)
    bias = np.asarray(bias, np.float32)
    dw = np.asarray(dense_w, np.float32)
    db = np.asarray(dense_b, np.float32)

    perm = _perm_cols()
    wd = rec + dw @ kern
    wfull = np.concatenate(
        [kern[:, perm], rec[:, perm], wd[:, perm], dw], axis=1
    ).astype(np.float16)  # [F, GC]

    bdec = bias + db @ kern
    emit_bias = bool(np.any(bias))
    emit_bdec = bool(np.any(bdec))
    emit_db = bool(np.any(db))

    # x^T per core: [t, 128, kt, b]
    eye = np.eye(BL, dtype=np.float16)
    in_maps = []
    for c in range(W):
        xc = x[c * BL : (c + 1) * BL, :t_warm, :]  # [BL, t, F]
        xt = np.ascontiguousarray(
            xc.transpose(1, 2, 0).reshape(t_warm, KT, 128, BL).transpose(0, 2, 1, 3)
        ).astype(np.float16)
        m = {
            "wsh": wfull[c * (F // W) : (c + 1) * (F // W)],
            "x_t": xt,
            "eye": eye,
        }
        if emit_bias:
            m["bias_bc"] = np.ascontiguousarray(
                np.broadcast_to(bias[perm], (BL, 4 * U))
            ).astype(np.float32)
        if emit_bdec:
            m["bdec_bc"] = np.ascontiguousarray(
                np.broadcast_to(bdec[perm], (BL, 4 * U))
            ).astype(np.float32)
        if emit_db:
            m["db_bc"] = np.ascontiguousarray(
                np.broadcast_to(db, (BL, F))
            ).astype(np.float32)
        in_maps.append(m)
    return in_maps, (emit_bias, emit_bdec, emit_db)


def kernel(
    inputs, kernel, rec_kernel, bias, dense_w, dense_b, t_warm=T,
    t_dec=OUT_STEPS - 1, trace=False
):
    in_maps, (emit_bias, emit_bdec, emit_db) = _prep_inputs(
        inputs, kernel, rec_kernel, bias, dense_w, dense_b, t_warm
    )
    nc = build_nc(
        t_warm=t_warm, t_dec=t_dec, emit_bias=emit_bias, emit_bdec=emit_bdec,
        emit_db=emit_db
    )
    _t0 = _time.time()
    res = run_bass_kernel_spmd(nc, in_maps, core_ids=list(range(W)), trace=trace)
    _wall_ns = int((_time.time() - _t0) * 1e9)
    # no NTFF hook under axon: fall back to wall clock of the SPMD dispatch
    _last_results["exec_time_ns"] = (
        res.exec_time_ns if res.exec_time_ns is not None else _wall_ns
    )
    _last_results["bass_results"] = res

    n_out = t_dec + 1
    preds = np.empty((B, n_out, F), np.float32)
    for c in range(W):
        o = res.results[c]["preds"].astype(np.float32)  # [n_out, UT, BL, 512]
        preds[c * BL : (c + 1) * BL] = o.transpose(2, 0, 1, 3).reshape(
            BL, n_out, F
        )
    return preds


# revision 30
# speedup vs baseline: 3.2015x; 3.2015x over previous
"""LSTM warmup+autoregressive-decode kernel for 8 Trainium2 NeuronCores.

Strategy (data-parallel over batch — zero recurring collectives):
  - Each core owns 64 of the 512 batch rows and computes the FULL LSTM for
    them. The per-step h AllGather of the old tensor-parallel design is gone;
    on this runtime every collective call costs ~0.6-0.9ms, which made the
    old design's 148 collective calls ~127ms of pure collective overhead.
  - Weights are shipped SHARDED (1/8 of the rows per core) and assembled
    on-device with ONE AllGather into a shared-DRAM buffer, then streamed
    from HBM every step (they exceed SBUF capacity).
  - Layout: batch on the stationary side (lhsT = x^T/h^T k-tiles [128, 64]),
    weights as the 512-wide moving operand -> full fp16 matmul rate; z lives
    as [64 batch, 512 gate-cols] PSUM tiles.
  - Gate columns are host-permuted to [u-tile][i|f|g|o][512] so the four
    gates of one u-tile arrive in consecutive PSUM tiles (4 live banks).
  - Decode folds the feedback path on host: z = h @ (rec + dense_w @ kernel),
    so decode streams one 32MB matrix + 8MB dense per step.
  - h is re-transposed each step with 16 PE-transposes (hidden under the
    weight stream); predictions are written untransposed [64, 512] tiles.

kernel(**inputs) takes the full unsharded inputs and returns [B, OUT, F].
"""

import sys, time as _time

for _p in ("/opt/trn_rl_repo", "/root/.axon_site/_ro/trn_rl_repo"):
    if _p not in sys.path:
        sys.path.insert(0, _p)

import os

import numpy as np

import concourse.bass as bass
import concourse.mybir as mybir
import concourse.tile as tile
from concourse import bacc
from concourse.bass import ts
from concourse.bass_utils import run_bass_kernel_spmd

B, T, F, U = 512, 48, 2048, 2048
OUT_STEPS = 24
W = 8  # cores
BL = B // W  # 64 batch rows per core
KT = F // 128  # 16 k-tiles over the x/h feature dim
UT = U // 512  # 4 u-tiles of 512 features
# weight m-tiles, each [128, KT, 512] (p, kt, n), shipped m-tile-major so a
# stream DMA is 128 fully-contiguous 16KB descriptors:
# q = 0..15 Kp | 16..31 Rp | 32..47 Wd | 48..51 dw | 52..55 pad
QT = 56  # 7 per core
OFF_K, OFF_R, OFF_D, OFF_W = 0, 16, 32, 48
FP16 = mybir.dt.float16
FP32 = mybir.dt.float32
AF = mybir.ActivationFunctionType

_last_results = {"exec_time_ns": None}


def build_nc(t_warm=T, t_dec=OUT_STEPS - 1, emit_bias=False, emit_bdec=False,
             emit_db=False):
    nc = bacc.Bacc("TRN2", target_bir_lowering=False, debug=False, num_devices=W)

    wsh_in = nc.dram_tensor(
        "wsh", [QT // W * 128 * KT, 512], FP16, kind="ExternalInput"
    )
    x_in = nc.dram_tensor(
        "x_t", [t_warm // 2 + t_warm % 2, 128, KT, 2 * BL], FP16,
        kind="ExternalInput"
    )
    eye_in = nc.dram_tensor("eye", [BL, BL], FP16, kind="ExternalInput")
    if emit_bias:
        bias_in = nc.dram_tensor("bias_bc", [BL, 4 * U], FP32, kind="ExternalInput")
    if emit_bdec:
        bdec_in = nc.dram_tensor("bdec_bc", [BL, 4 * U], FP32, kind="ExternalInput")
    if emit_db:
        db_in = nc.dram_tensor("db_bc", [BL, F], FP32, kind="ExternalInput")
    p_out = nc.dram_tensor(
        "preds", [t_dec + 1, UT, BL, 512], FP16, kind="ExternalOutput"
    )

    npair = t_warm // 2
    nodd = t_warm % 2

    with tile.TileContext(nc) as tc:
        with (
            tc.tile_pool(name="wstr", bufs=3) as wstr,
            tc.tile_pool(name="cpool", bufs=1) as cpool,
            tc.tile_pool(name="hbufs", bufs=2) as hbufs,
            tc.tile_pool(name="xbufs", bufs=2) as xbufs,
            tc.tile_pool(name="zxbufs", bufs=2) as zxbufs,
            tc.tile_pool(name="gtmp", bufs=2) as gtmp,
            tc.tile_pool(name="outp", bufs=4) as outp,
            tc.tile_pool(name="zps", bufs=4, space="PSUM") as zps,
            tc.tile_pool(name="tps", bufs=2, space="PSUM") as tps,
            tc.tile_pool(name="pps", bufs=2, space="PSUM") as pps,
            tc.tile_pool(name="agin", bufs=1, space="DRAM") as agin,
            tc.tile_pool(name="agout", bufs=1, space="DRAM") as agout,
            tc.tile_pool(name="zxd", bufs=1, space="DRAM") as zxdp,
        ):
            # --- one-time weight assembly: ship shards, AllGather once ---
            shrows = QT // W * 128 * KT
            wb = agin.tile([shrows, 512], FP16, tag="wagin")
            nc.sync.dma_start(wb[:], wsh_in[:])
            wg = agout.tile([W * shrows, 512], FP16, addr_space="Shared", tag="wagout")
            if os.environ.get("SKIP_AG"):
                nc.sync.dma_start(wg[0:shrows, :], wb[:])
            else:
                nc.gpsimd.collective_compute(
                    "AllGather",
                    mybir.AluOpType.bypass,
                    replica_groups=[list(range(W))],
                    ins=[wb[:].opt()],
                    outs=[wg[:].opt()],
                )
            wg4 = wg.rearrange("(q p kt) n -> q p kt n", p=128, kt=KT)
            wg5 = wg.rearrange("(qp two p kt) n -> qp p two kt n", two=2, p=128, kt=KT)

            def wtile(off, m):
                """Weight stream source: [128, KT, 512], fully contiguous per
                partition (16KB runs)."""
                return wg4[off + m]

            def wpair(off, m):
                """Two consecutive m-tiles in one DMA: [128, 2, KT, 512]."""
                assert (off + m) % 2 == 0
                return wg5[(off + m) // 2]

            eye_sb = cpool.tile([BL, BL], FP16, tag="eye")
            nc.sync.dma_start(eye_sb[:], eye_in[:])
            bias_sb = None
            bdec_sb = None
            db_sb = None
            if emit_bias:
                bias_sb = cpool.tile([BL, 4 * U], FP32, tag="biasbc")
                nc.sync.dma_start(bias_sb[:], bias_in[:])
            if emit_bdec:
                bdec_sb = cpool.tile([BL, 4 * U], FP32, tag="bdecbc")
                nc.sync.dma_start(bdec_sb[:], bdec_in[:])
            if emit_db:
                db_sb = cpool.tile([BL, F], FP32, tag="dbbc")
                nc.sync.dma_start(db_sb[:], db_in[:])

            c_st = [
                cpool.tile([BL, 512], FP32, tag=f"c{u}", name=f"c_st{u}")
                for u in range(UT)
            ]
            for cs in c_st:
                nc.vector.memset(cs[:], 0.0)

            # --- Zx phase: z_x[t] = x_t @ kernel for every warmup step, with
            # K streamed once (in 2 halves of 8 m-tiles kept in SBUF) and two
            # timesteps packed into the 128-wide stationary operand.
            zxd = zxdp.tile(
                [npair + nodd, 128, 16, 512], FP16, tag="zxd", name="zxdram"
            )
            for mh in range(4):
                khalf = [
                    wstr.tile([128, 2, KT, 512], FP16, tag="wstr", name=f"kh{mh}_{j}")
                    for j in range(2)
                ]
                for j in range(2):
                    nc.sync.dma_start(khalf[j][:], wpair(OFF_K, mh * 4 + 2 * j))
                for pr in range(npair + nodd):
                    x2 = zxbufs.tile([128, KT, 2 * BL], FP16, tag="x2")
                    nc.sync.dma_start(x2[:], x_in[pr])
                    mw = 2 * BL if 2 * pr + 1 < t_warm else BL
                    for m in range(4):
                        zx = zps.tile([mw, 512], FP32, tag="z", name=f"zx{mh}_{pr}_{m}")
                        for kt in range(KT):
                            nc.tensor.matmul(
                                zx[:],
                                x2[:, kt, 0:mw],
                                khalf[m // 2][:, m % 2, kt, :],
                                start=(kt == 0),
                                stop=(kt == KT - 1),
                            )
                        zxs = outp.tile([mw, 512], FP16, tag="zxs")
                        nc.scalar.activation(zxs[:], zx[:], AF.Identity)
                        nc.sync.dma_start(zxd[pr, 0:mw, mh * 4 + m, :], zxs[:])

            def lstm_step(emit_zp, step_bias):
                """One LSTM step. emit_zp(ut, g) -> list of operand APs
                ([BL, 512], PSUM or SBUF) to be summed for that gate slice.
                Returns hT_next ([128, KT, BL] fp16)."""
                hT_next = hbufs.tile([128, KT, BL], FP16, tag="hT")
                for ut in range(UT):
                    acts = []
                    for g, (fn, tag) in enumerate(
                        [
                            (AF.Sigmoid, "si"),
                            (AF.Sigmoid, "sf"),
                            (AF.Tanh, "tg"),
                            (AF.Sigmoid, "so"),
                        ]
                    ):
                        srcs = emit_zp(ut, g)
                        if step_bias is not None:
                            srcs.append(step_bias[:, ts(ut * 4 + g, 512)])
                        src = srcs[0]
                        for extra in srcs[1:]:
                            zb = gtmp.tile([BL, 512], FP32, tag="zb")
                            nc.vector.tensor_tensor(
                                zb[:], src, extra, mybir.AluOpType.add
                            )
                            src = zb[:]
                        a = gtmp.tile([BL, 512], FP16, tag=tag)
                        nc.scalar.activation(a[:], src, fn)
                        acts.append(a)
                    si, sf, tg, so = acts
                    t1 = gtmp.tile([BL, 512], FP32, tag="t1")
                    t2 = gtmp.tile([BL, 512], FP32, tag="t2")
                    nc.vector.tensor_tensor(
                        t1[:], sf[:], c_st[ut][:], mybir.AluOpType.mult
                    )
                    nc.vector.tensor_tensor(t2[:], si[:], tg[:], mybir.AluOpType.mult)
                    nc.vector.tensor_tensor(
                        c_st[ut][:], t1[:], t2[:], mybir.AluOpType.add
                    )
                    tcs = gtmp.tile([BL, 512], FP16, tag="tc")
                    nc.scalar.activation(tcs[:], c_st[ut][:], AF.Tanh)
                    hn = gtmp.tile([BL, 512], FP16, tag="hn")
                    nc.vector.tensor_tensor(hn[:], so[:], tcs[:], mybir.AluOpType.mult)
                    # transpose h_new [64, 512] -> 4 tiles of [128, 64]
                    for j in range(4):
                        tp = tps.tile([128, BL], FP16, tag="tp")
                        nc.tensor.transpose(
                            tp[:], hn[:, ts(j, 128)], eye_sb[:]
                        )
                        nc.scalar.activation(
                            hT_next[:, ut * 4 + j, :], tp[:], AF.Identity
                        )
                return hT_next

            def emit_pred(hT, t_idx):
                """pred = h @ dense_w (+ db): 4 tiles of [64, 512] to DRAM."""
                pb_ = {}
                for ft in range(UT):
                    if ft % 2 == 0:
                        pt = wstr.tile(
                            [128, 2, KT, 512], FP16, tag="wstr",
                            name=f"dw{t_idx}_{ft}"
                        )
                        nc.sync.dma_start(pt[:], wpair(OFF_W, ft))
                        pb_["t"] = pt
                    dstr = pb_["t"]
                    pp = pps.tile([BL, 512], FP32, tag="pp")
                    for kt in range(KT):
                        nc.tensor.matmul(
                            pp[:],
                            hT[:, kt, :],
                            dstr[:, ft % 2, kt, :],
                            start=(kt == 0),
                            stop=(kt == KT - 1),
                        )
                    po = outp.tile([BL, 512], FP16, tag="po")
                    if db_sb is not None:
                        pb = gtmp.tile([BL, 512], FP32, tag="pb")
                        nc.vector.tensor_tensor(
                            pb[:], pp[:], db_sb[:, ts(ft, 512)], mybir.AluOpType.add
                        )
                        nc.scalar.activation(po[:], pb[:], AF.Identity)
                    else:
                        nc.scalar.activation(po[:], pp[:], AF.Identity)
                    nc.sync.dma_start(p_out[t_idx, ft], po[:])

            # ---------------- warmup ----------------
            hT = None
            for t in range(t_warm):
                zxcur = xbufs.tile([BL, 16, 512], FP16, tag="zxc")
                off = (t % 2) * BL
                nc.sync.dma_start(zxcur[:], zxd[t // 2, off : off + BL])

                pairbox = {}

                def z_mm(ut, g, zxcur=zxcur, hT=hT, first=(t == 0), pairbox=pairbox):
                    m = ut * 4 + g
                    if first:
                        return [zxcur[:, m, :]]
                    if m % 2 == 0:
                        pt = wstr.tile([128, 2, KT, 512], FP16, tag="wstr")
                        nc.sync.dma_start(pt[:], wpair(OFF_R, m))
                        pairbox["t"] = pt
                    rstr = pairbox["t"]
                    zp = zps.tile([BL, 512], FP32, tag="z")
                    for kt in range(KT):
                        nc.tensor.matmul(
                            zp[:],
                            hT[:, kt, :],
                            rstr[:, m % 2, kt, :],
                            start=(kt == 0),
                            stop=(kt == KT - 1),
                        )
                    return [zp[:], zxcur[:, m, :]]

                hT = lstm_step(z_mm, bias_sb[:] if emit_bias else None)

            emit_pred(hT, 0)

            # ---------------- decode ----------------
            for t in range(t_dec):

                pairbox = {}

                def z_mm(ut, g, hT=hT, pairbox=pairbox):
                    m = ut * 4 + g
                    if m % 2 == 0:
                        pt = wstr.tile([128, 2, KT, 512], FP16, tag="wstr")
                        nc.sync.dma_start(pt[:], wpair(OFF_D, m))
                        pairbox["t"] = pt
                    dstr = pairbox["t"]
                    zp = zps.tile([BL, 512], FP32, tag="z")
                    for kt in range(KT):
                        nc.tensor.matmul(
                            zp[:],
                            hT[:, kt, :],
                            dstr[:, m % 2, kt, :],
                            start=(kt == 0),
                            stop=(kt == KT - 1),
                        )
                    return [zp[:]]

                hT = lstm_step(z_mm, bdec_sb[:] if emit_bdec else None)
                emit_pred(hT, t + 1)

    nc.compile()
    return nc


def _perm_cols():
    # permuted gate cols: [u-tile][i|f|g|o][512]
    m = np.arange(4 * U)
    ut, rem = m // (4 * 512), m % (4 * 512)
    g, j = rem // 512, rem % 512
    return g * U + ut * 512 + j


def _prep_inputs(inputs, kernel, rec_kernel, bias, dense_w, dense_b, t_warm):
    x = np.asarray(inputs, np.float32)
    kern = np.asarray(kernel, np.float32)
    rec = np.asarray(rec_kernel, np.float32)
    bias = np.asarray(bias, np.float32)
    dw = np.asarray(dense_w, np.float32)
    db = np.asarray(dense_b, np.float32)

    perm = _perm_cols()
    wd = rec + dw @ kern
    # m-tile-major weight blob: for each [2048, 512] column slice, relayout
    # to [128 p, 16 kt, 512] so the device stream DMA is 16KB-contiguous.
    def mtiles(mat):  # [F, 512*nm] -> [nm, 128, KT, 512]
        nm = mat.shape[1] // 512
        return (
            mat.astype(np.float16)
            .reshape(F // 128, 128, nm, 512)
            .transpose(2, 1, 0, 3)
        )  # [nm, p, kt, n]

    wtiles = np.concatenate(
        [
            mtiles(kern[:, perm]),
            mtiles(rec[:, perm]),
            mtiles(wd[:, perm]),
            mtiles(dw),
            np.zeros((QT - 52, 128, KT, 512), np.float16),
        ],
        axis=0,
    )  # [QT, 128, KT, 512]

    bdec = bias + db @ kern
    emit_bias = bool(np.any(bias))
    emit_bdec = bool(np.any(bdec))
    emit_db = bool(np.any(db))

    # x^T per core, timestep-paired: [pair, 128, kt, 2*BL] where the free
    # axis holds [t=2p batch | t=2p+1 batch] (odd tail zero-padded)
    eye = np.eye(BL, dtype=np.float16)
    npair_tot = t_warm // 2 + t_warm % 2
    in_maps = []
    for c in range(W):
        xc = x[c * BL : (c + 1) * BL, :t_warm, :]  # [BL, t, F]
        xt1 = xc.transpose(1, 2, 0).reshape(t_warm, KT, 128, BL).transpose(0, 2, 1, 3)
        xt = np.zeros((npair_tot, 128, KT, 2 * BL), np.float16)
        xt[:, :, :, 0:BL] = xt1[0::2]
        xt[: t_warm // 2, :, :, BL : 2 * BL] = xt1[1::2]
        m = {
            "wsh": np.ascontiguousarray(
                wtiles[c * (QT // W) : (c + 1) * (QT // W)]
            ).reshape(QT // W * 128 * KT, 512),
            "x_t": xt,
            "eye": eye,
        }
        if emit_bias:
            m["bias_bc"] = np.ascontiguousarray(
                np.broadcast_to(bias[perm], (BL, 4 * U))
            ).astype(np.float32)
        if emit_bdec:
            m["bdec_bc"] = np.ascontiguousarray(
                np.broadcast_to(bdec[perm], (BL, 4 * U))
            ).astype(np.float32)
        if emit_db:
            m["db_bc"] = np.ascontiguousarray(
                np.broadcast_to(db, (BL, F))
            ).astype(np.float32)
        in_maps.append(m)
    return in_maps, (emit_bias, emit_bdec, emit_db)


def kernel(
    inputs, kernel, rec_kernel, bias, dense_w, dense_b, t_warm=T,
    t_dec=OUT_STEPS - 1, trace=False
):
    in_maps, (emit_bias, emit_bdec, emit_db) = _prep_inputs(
        inputs, kernel, rec_kernel, bias, dense_w, dense_b, t_warm
    )
    nc = build_nc(
        t_warm=t_warm, t_dec=t_dec, emit_bias=emit_bias, emit_bdec=emit_bdec,
        emit_db=emit_db
    )
    _t0 = _time.time()
    res = run_bass_kernel_spmd(nc, in_maps, core_ids=list(range(W)), trace=trace)
    _wall_ns = int((_time.time() - _t0) * 1e9)
    # no NTFF hook under axon: fall back to wall clock of the SPMD dispatch
    _last_results["exec_time_ns"] = (
        res.exec_time_ns if res.exec_time_ns is not None else _wall_ns
    )
    _last_results["bass_results"] = res

    n_out = t_dec + 1
    preds = np.empty((B, n_out, F), np.float32)
    for c in range(W):
        o = res.results[c]["preds"].astype(np.float32)  # [n_out, UT, BL, 512]
        preds[c * BL : (c + 1) * BL] = o.transpose(2, 0, 1, 3).reshape(
            BL, n_out, F
        )
    return preds
